# revision 8
# baseline (speedup 1.0000x reference)
"""Trainium2 Bass kernel for nn_ModelDEP (biaffine-ish dependency parser loss).

Contract: kernel(**inputs) takes FULL unsharded numpy inputs (as produced by
reference.setup_inputs()) and returns the FULL output (scalar f32 loss).

Strategy (hardcoded, self-contained):
  - Data parallel over batch: B=16 examples -> 8 cores x 2 examples.
  - Per example, on device:
      hidden_T = relu(W1.T @ ctx_T + b1)            [256h x 128i] (h on partitions)
      cwr_T    = [root | hidden_T]                  [256h x 129j]
      ha_T     = Wa.T @ hidden_T + bp               [256 x 128]   (bp folded here)
      cbb_T    = Wb.T @ cwr_T                       [256 x 129]
      arc[i,j] = W_arc . relu(ha_T[:,i] + cbb_T[:,j])
        - per (j, h-chunk): one fused (add bias, max 0) op -> bf16 [128,128] tile
          (split between DVE tensor_scalar and ACT activation-relu-with-bias)
        - TensorE: lhsT = pairs tile (stationary, bf16 FWL), rhs = W_arc chunk
          -> PSUM column [128i, 1], accumulated over the 2 h-chunks
      arc CE via logsumexp over j (reduce_max / exp+accum / ln) and gold logit
        via (iota == gold) * logits with fused accumulate.
      label path: cbb rows in [j,h] layout -> internal DRAM; indirect-DMA gather
        rows at gold arcs; PE transpose; sel_T = relu(ha_T + gathered.T);
        label logits = sel_T.T @ W_lab + b_lab; logsumexp + gold as above.
      per-token ce = arc_ce + lab_ce -> [128, 2] output per core.
  - Host: mask by sentence length, global sum, /denom, *0.5.
"""

import sys
import numpy as np

for _p in ("/opt/trn_rl_repo", "/root/.axon_site/_ro/trn_rl_repo"):
    if _p not in sys.path:
        sys.path.append(_p)

import ml_dtypes

import concourse.bass as bass
from concourse import bacc
import concourse.mybir as mybir
import concourse.tile as tile
from concourse.bass import IndirectOffsetOnAxis
from concourse.bass_utils import run_bass_kernel_spmd
from concourse.masks import make_identity
from concourse.tile_rust import add_dep_helper

BF16 = mybir.dt.bfloat16
F32 = mybir.dt.float32
I32 = mybir.dt.int32
AF = mybir.ActivationFunctionType
ALU = mybir.AluOpType

B, L, D, H, TAGS = 16, 128, 512, 256, 45
NC_CORES = 8
NB = B // NC_CORES  # examples per core
J = L + 1  # head candidates (root + tokens)
HC = H // 128  # h chunks
DC = D // 128  # d chunks

_nb = ml_dtypes.bfloat16

_cached = {}

# j-loop relu engine split by (j*HC+hc) % 10: measured rates
# DVE ~163ns, GPSIMD ~?ns, ACT ~326ns per [128,128] tile
RELU_DVE = {0, 2, 4, 6, 8}
RELU_ACT = {3, 9}
# remaining {1, 5, 7} go to GPSIMD


def _build_program():
    nc = bacc.Bacc("TRN2", target_bir_lowering=False, debug=False, num_devices=NC_CORES)

    # ---- I/O ----
    ctx_d = nc.dram_tensor("ctx_bf", [NB, 128, DC, 128], BF16, kind="ExternalInput")
    w1_d = nc.dram_tensor("w1_bf", [128, DC, H], BF16, kind="ExternalInput")
    wa_d = nc.dram_tensor("wa_bf", [128, HC, H], BF16, kind="ExternalInput")
    wb_d = nc.dram_tensor("wb_bf", [128, HC, H], BF16, kind="ExternalInput")
    warc_d = nc.dram_tensor("warc_bf", [128, HC, 1], BF16, kind="ExternalInput")
    wlab_d = nc.dram_tensor("wlab_bf", [128, HC, TAGS], BF16, kind="ExternalInput")
    b1_d = nc.dram_tensor("b1_f", [128, HC, 1], F32, kind="ExternalInput")
    bp_d = nc.dram_tensor("bp_f", [128, HC, 1], F32, kind="ExternalInput")
    blab_d = nc.dram_tensor("blab_f", [1, TAGS], F32, kind="ExternalInput")
    ones_d = nc.dram_tensor("ones_f", [1, 128], F32, kind="ExternalInput")
    root_d = nc.dram_tensor("root_bf", [128, HC, 1], BF16, kind="ExternalInput")
    iota_d = nc.dram_tensor("iota_f", [128, J], F32, kind="ExternalInput")
    arcs_d = nc.dram_tensor("arcs_f", [128, NB], F32, kind="ExternalInput")
    labs_d = nc.dram_tensor("labs_f", [128, NB], F32, kind="ExternalInput")
    gidx_d = nc.dram_tensor("gidx_i", [128, NB], I32, kind="ExternalInput")
    ce_d = nc.dram_tensor("ce_out", [128, NB], F32, kind="ExternalOutput")
    cbb_ds = [nc.dram_tensor(f"cbb_scratch{b}", [J, H], F32) for b in range(NB)]

    with tile.TileContext(nc) as tc:
        with (
            tc.tile_pool(name="consts", bufs=1) as consts,
            tc.tile_pool(name="bpool", bufs=2) as bpool,
            tc.tile_pool(name="small", bufs=4) as small,
            tc.tile_pool(name="pairs", bufs=16) as pairs_pool,
            tc.tile_pool(name="ps_big", bufs=2, space="PSUM") as ps_big,
            tc.tile_pool(name="ps_work", bufs=2, space="PSUM") as ps_work,
            tc.tile_pool(name="ps_lab", bufs=1, space="PSUM") as ps_lab,
        ):
            # ---- load constants ----
            w1_sb = consts.tile([128, DC, H], BF16)
            nc.sync.dma_start(out=w1_sb[:], in_=ctx_ap(w1_d))
            wa_sb = consts.tile([128, HC, H], BF16)
            nc.sync.dma_start(out=wa_sb[:], in_=ctx_ap(wa_d))
            wb_sb = consts.tile([128, HC, H], BF16)
            nc.sync.dma_start(out=wb_sb[:], in_=ctx_ap(wb_d))
            warc_sb = consts.tile([128, HC, 1], BF16)
            nc.sync.dma_start(out=warc_sb[:], in_=ctx_ap(warc_d))
            wlab_sb = consts.tile([128, HC, TAGS], BF16)
            nc.sync.dma_start(out=wlab_sb[:], in_=ctx_ap(wlab_d))
            b1_sb = consts.tile([128, HC, 1], F32)
            nc.sync.dma_start(out=b1_sb[:], in_=ctx_ap(b1_d))
            bp_sb = consts.tile([128, HC, 1], F32)
            nc.sync.dma_start(out=bp_sb[:], in_=ctx_ap(bp_d))
            blab_sb = consts.tile([1, TAGS], F32)
            nc.sync.dma_start(out=blab_sb[:], in_=ctx_ap(blab_d))
            ones_sb = consts.tile([1, 128], F32)
            nc.sync.dma_start(out=ones_sb[:], in_=ctx_ap(ones_d))
            root_sb = consts.tile([128, HC, 1], BF16)
            nc.sync.dma_start(out=root_sb[:], in_=ctx_ap(root_d))
            iota_sb = consts.tile([128, J], F32)
            nc.sync.dma_start(out=iota_sb[:], in_=ctx_ap(iota_d))
            arcs_sb = consts.tile([128, NB], F32)
            nc.sync.dma_start(out=arcs_sb[:], in_=ctx_ap(arcs_d))
            labs_sb = consts.tile([128, NB], F32)
            nc.sync.dma_start(out=labs_sb[:], in_=ctx_ap(labs_d))
            gidx_sb = consts.tile([128, NB], I32)
            nc.sync.dma_start(out=gidx_sb[:], in_=ctx_ap(gidx_d))
            ident_sb = consts.tile([128, 128], F32)
            make_identity(nc, ident_sb[:])
            ce_sb = consts.tile([128, NB], F32)

            for b in range(NB):
                # ---- ctx load (host pre-transposed): [128 d, DC, 128 i]
                ctxT = bpool.tile([128, DC, 128], BF16, tag="ctxT")
                nc.sync.dma_start(out=ctxT[:], in_=ctx_d.ap()[b])
                # ---- hidden (into cwr cols 1..128) ----
                cwrT = bpool.tile([128, HC, J], BF16, tag="cwrT")
                nc.vector.tensor_copy(cwrT[:, :, 0:1], root_sb[:])
                for hc in range(HC):
                    phw = ps_work.tile([128, H], F32, tag="work")
                    ph = phw[:, :128]
                    for dc in range(DC):
                        nc.tensor.matmul(
                            ph[:],
                            lhsT=w1_sb[:, dc, hc * 128 : (hc + 1) * 128],
                            rhs=ctxT[:, dc, :],
                            start=(dc == 0),
                            stop=(dc == DC - 1),
                        )
                    nc.scalar.activation(
                        cwrT[:, hc, 1:J], ph[:], AF.Relu, bias=b1_sb[:, hc, :]
                    )
                # ---- ha_T (+bp folded) ----
                haT = bpool.tile([128, HC, 128], BF16, tag="haT")
                for ac in range(HC):
                    paw = ps_work.tile([128, H], F32, tag="work")
                    pa = paw[:, :128]
                    for hc in range(HC):
                        nc.tensor.matmul(
                            pa[:],
                            lhsT=wa_sb[:, hc, ac * 128 : (ac + 1) * 128],
                            rhs=cwrT[:, hc, 1:J],
                            start=(hc == 0),
                            stop=(hc == HC - 1),
                        )
                    nc.vector.tensor_scalar(
                        out=haT[:, ac, :],
                        in0=pa[:],
                        scalar1=bp_sb[:, ac, :],
                        scalar2=None,
                        op0=ALU.add,
                    )
                # ---- cbb_T [128, 2, 129] f32 ----
                cbbT = bpool.tile([128, HC, J], F32, tag="cbbT")
                for bc in range(HC):
                    pc = ps_big.tile([128, J], F32, tag="pcb")
                    for hc in range(HC):
                        nc.tensor.matmul(
                            pc[:],
                            lhsT=wb_sb[:, hc, bc * 128 : (bc + 1) * 128],
                            rhs=cwrT[:, hc, :],
                            start=(hc == 0),
                            stop=(hc == HC - 1),
                        )
                    nc.scalar.copy(cbbT[:, bc, :], pc[:])
                # ---- cbb in [j, h] layout -> DRAM (for the gather) ----
                cj = bpool.tile([128, H], F32, tag="cj")
                pj = ps_work.tile([128, H], F32, tag="work")
                for hc in range(HC):
                    nc.tensor.matmul(
                        pj[:],
                        lhsT=cwrT[:, hc, 0:128],
                        rhs=wb_sb[:, hc, :],
                        start=(hc == 0),
                        stop=(hc == HC - 1),
                    )
                nc.vector.tensor_copy(cj[:], pj[:])
                st1 = nc.sync.dma_start(
                    out=cbb_ds[b].ap()[0:128, :], in_=cj[:]
                )
                cjl = bpool.tile([1, H], F32, tag="cjl")
                pjl = ps_lab.tile([1, H], F32, tag="pjl")
                for hc in range(HC):
                    nc.tensor.matmul(
                        pjl[:],
                        lhsT=cwrT[:, hc, 128:129],
                        rhs=wb_sb[:, hc, :],
                        start=(hc == 0),
                        stop=(hc == HC - 1),
                    )
                nc.vector.tensor_copy(cjl[:], pjl[:])
                st2 = nc.sync.dma_start(
                    out=cbb_ds[b].ap()[128:J, :], in_=cjl[:]
                )
                # ---- gather cbb rows at gold arcs ----
                csel = bpool.tile([128, H], F32, tag="csel")
                g = nc.gpsimd.indirect_dma_start(
                    out=csel[:],
                    out_offset=None,
                    in_=cbb_ds[b].ap(),
                    in_offset=IndirectOffsetOnAxis(ap=gidx_sb[:, b : b + 1], axis=0),
                )
                add_dep_helper(g.ins, st1.ins, sync=True, reason="cbb store->gather")
                add_dep_helper(g.ins, st2.ins, sync=True, reason="cbb store->gather")

                # ---- the quadratic j-loop ----
                arc_ps = ps_big.tile([128, J], F32, tag="arc")
                for j in range(J):
                    for hc in range(HC):
                        pt = pairs_pool.tile([128, 128], BF16, tag="pairs")
                        k = (j * HC + hc) % 10
                        if k in RELU_ACT:
                            nc.scalar.activation(
                                pt[:],
                                haT[:, hc, :],
                                AF.Relu,
                                bias=cbbT[:, hc, j : j + 1],
                            )
                        else:
                            eng = nc.vector if k in RELU_DVE else nc.gpsimd
                            eng.tensor_scalar(
                                out=pt[:],
                                in0=haT[:, hc, :],
                                scalar1=cbbT[:, hc, j : j + 1],
                                scalar2=0.0,
                                op0=ALU.add,
                                op1=ALU.max,
                            )
                        nc.tensor.matmul(
                            arc_ps[:, j : j + 1],
                            lhsT=pt[:],
                            rhs=warc_sb[:, hc, :],
                            start=(hc == 0),
                            stop=(hc == HC - 1),
                        )

                # ---- arc logsumexp + gold ----
                negm = small.tile([128, 1], F32, tag="negm")
                nc.vector.tensor_reduce(
                    negm[:], arc_ps[:], axis=mybir.AxisListType.X, op=ALU.max,
                    negate=True,
                )
                et = bpool.tile([128, J], F32, tag="et")
                es = small.tile([128, 1], F32, tag="es")
                nc.scalar.activation(
                    et[:], arc_ps[:], AF.Exp, bias=negm[:], accum_out=es[:]
                )
                lns = small.tile([128, 1], F32, tag="lns")
                nc.scalar.activation(lns[:], es[:], AF.Ln)
                golda = small.tile([128, 1], F32, tag="golda")
                sc2 = bpool.tile([128, J], F32, tag="sc2")
                nc.vector.scalar_tensor_tensor(
                    out=sc2[:],
                    in0=iota_sb[:],
                    scalar=arcs_sb[:, b : b + 1],
                    op0=ALU.is_equal,
                    in1=arc_ps[:],
                    op1=ALU.mult,
                    accum_out=golda[:],
                )
                cea = small.tile([128, 1], F32, tag="cea")
                nc.vector.tensor_sub(cea[:], lns[:], negm[:])
                nc.vector.tensor_sub(cea[:], cea[:], golda[:])

                # ---- label path ----
                selT = bpool.tile([128, HC, 128], BF16, tag="selT")
                for hc in range(HC):
                    ptrw = ps_work.tile([128, H], F32, tag="work")
                    ptr = ptrw[:, :128]
                    nc.tensor.transpose(
                        ptr[:], csel[:, hc * 128 : (hc + 1) * 128], ident_sb[:]
                    )
                    tmp = bpool.tile([128, 128], F32, tag="seltmp")
                    nc.vector.tensor_add(tmp[:], ptr[:], haT[:, hc, :])
                    nc.vector.tensor_scalar(
                        out=selT[:, hc, :], in0=tmp[:], scalar1=0.0, op0=ALU.max,
                        scalar2=None,
                    )
                lab_ps = ps_lab.tile([128, TAGS], F32, tag="lab")
                for hc in range(HC):
                    nc.tensor.matmul(
                        lab_ps[:],
                        lhsT=selT[:, hc, :],
                        rhs=wlab_sb[:, hc, :],
                        start=(hc == 0),
                        stop=False,
                    )
                nc.tensor.matmul(
                    lab_ps[:], lhsT=ones_sb[:], rhs=blab_sb[:], start=False, stop=True
                )
                negml = small.tile([128, 1], F32, tag="negml")
                nc.vector.tensor_reduce(
                    negml[:], lab_ps[:], axis=mybir.AxisListType.X, op=ALU.max,
                    negate=True,
                )
                etl = bpool.tile([128, TAGS], F32, tag="etl")
                esl = small.tile([128, 1], F32, tag="esl")
                nc.scalar.activation(
                    etl[:], lab_ps[:], AF.Exp, bias=negml[:], accum_out=esl[:]
                )
                lnsl = small.tile([128, 1], F32, tag="lnsl")
                nc.scalar.activation(lnsl[:], esl[:], AF.Ln)
                goldl = small.tile([128, 1], F32, tag="goldl")
                sc2l = bpool.tile([128, TAGS], F32, tag="sc2l")
                nc.vector.scalar_tensor_tensor(
                    out=sc2l[:],
                    in0=iota_sb[:, :TAGS],
                    scalar=labs_sb[:, b : b + 1],
                    op0=ALU.is_equal,
                    in1=lab_ps[:],
                    op1=ALU.mult,
                    accum_out=goldl[:],
                )
                cel = small.tile([128, 1], F32, tag="cel")
                nc.vector.tensor_sub(cel[:], lnsl[:], negml[:])
                nc.vector.tensor_sub(cel[:], cel[:], goldl[:])

                nc.vector.tensor_add(ce_sb[:, b : b + 1], cea[:], cel[:])

            nc.sync.dma_start(out=ce_d.ap(), in_=ce_sb[:])

    nc.compile()
    return nc


def ctx_ap(d):
    return d.ap()


def _prep_in_maps(inputs):
    ctx = np.asarray(inputs["contextualized"], np.float32)
    arcs = np.asarray(inputs["desired_arcs"], np.int32)
    labs = np.asarray(inputs["desired_labels"], np.int32)
    W1 = np.asarray(inputs["W1"], np.float32)
    b1 = np.asarray(inputs["b1"], np.float32)
    root = np.asarray(inputs["root"], np.float32)
    Wp = np.asarray(inputs["Wp"], np.float32)
    bp = np.asarray(inputs["bp"], np.float32)
    W_arc = np.asarray(inputs["W_arc"], np.float32)
    W_lab = np.asarray(inputs["W_lab"], np.float32)
    b_lab = np.asarray(inputs["b_lab"], np.float32)

    def chunked(w, nch):  # [nch*128, X] -> [128, nch, X]
        return np.ascontiguousarray(
            w.reshape(nch, 128, -1).transpose(1, 0, 2)
        )

    w1_bf = chunked(W1, DC).astype(_nb)
    wa_bf = chunked(Wp[:H], HC).astype(_nb)
    wb_bf = chunked(Wp[H:], HC).astype(_nb)
    warc_bf = chunked(W_arc, HC).astype(_nb)
    wlab_bf = chunked(W_lab, HC).astype(_nb)
    b1_f = chunked(b1[:, None], HC)
    bp_f = chunked(bp[:, None], HC)
    root_bf = chunked(root[:, None], HC).astype(_nb)
    blab_f = np.ascontiguousarray(b_lab[None, :], dtype=np.float32)
    ones_f = np.ones((1, 128), np.float32)
    iota_f = np.broadcast_to(
        np.arange(J, dtype=np.float32)[None, :], (128, J)
    ).copy()

    in_maps = []
    for c in range(NC_CORES):
        bs = slice(c * NB, (c + 1) * NB)
        arcs_c = arcs[bs]  # [NB, 128]
        in_maps.append(
            {
                "ctx_bf": np.ascontiguousarray(
                    ctx[bs].reshape(NB, L, DC, 128).transpose(0, 3, 2, 1)
                ).astype(_nb),
                "w1_bf": w1_bf,
                "wa_bf": wa_bf,
                "wb_bf": wb_bf,
                "warc_bf": warc_bf,
                "wlab_bf": wlab_bf,
                "b1_f": b1_f,
                "bp_f": bp_f,
                "blab_f": blab_f,
                "ones_f": ones_f,
                "root_bf": root_bf,
                "iota_f": iota_f,
                "arcs_f": np.ascontiguousarray(arcs_c.T, np.float32),
                "labs_f": np.ascontiguousarray(labs[bs].T, np.float32),
                "gidx_i": np.ascontiguousarray(arcs_c.T).astype(np.int32),
            }
        )
    return in_maps


def kernel(**inputs) -> np.ndarray:
    if "nc" not in _cached:
        _cached["nc"] = _build_program()
    nc = _cached["nc"]
    in_maps = _prep_in_maps(inputs)
    res = run_bass_kernel_spmd(nc, in_maps, list(range(NC_CORES)))
    ce = np.concatenate([r["ce_out"] for r in res.results], axis=1)  # [128, B]
    lens = np.asarray(inputs["sentence_lengths"], np.int32)  # [B]
    mask = (np.arange(L)[None, :] < lens[:, None]).astype(np.float32)  # [B, L]
    total = float(np.sum(ce.T.astype(np.float64) * mask))
    denom = max(float(mask.sum()), 1.0)
    return np.array(0.5 * total / denom, dtype=np.float32)


# revision 9
# speedup vs baseline: 3.4794x; 3.4794x over previous
"""Trainium2 Bass kernel for nn_ModelDEP (biaffine-ish dependency parser loss).

Contract: kernel(**inputs) takes FULL unsharded numpy inputs (as produced by
reference.setup_inputs()) and returns the FULL output (scalar f32 loss).

Strategy (hardcoded, self-contained):
  - Data parallel over batch: B=16 examples -> 8 cores x 2 examples.
  - Per example, on device:
      hidden_T = relu(W1.T @ ctx_T + b1)            [256h x 128i] (h on partitions)
      cwr_T    = [root | hidden_T]                  [256h x 129j]
      ha_T     = Wa.T @ hidden_T + bp               [256 x 128]   (bp folded here)
      cbb_T    = Wb.T @ cwr_T                       [256 x 129]
      arc[i,j] = W_arc . relu(ha_T[:,i] + cbb_T[:,j])
        - per (j, h-chunk): one fused (add bias, max 0) op -> bf16 [128,128] tile
          (split between DVE tensor_scalar and ACT activation-relu-with-bias)
        - TensorE: lhsT = pairs tile (stationary, bf16 FWL), rhs = W_arc chunk
          -> PSUM column [128i, 1], accumulated over the 2 h-chunks
      arc CE via logsumexp over j (reduce_max / exp+accum / ln) and gold logit
        via (iota == gold) * logits with fused accumulate.
      label path: cbb rows in [j,h] layout -> internal DRAM; indirect-DMA gather
        rows at gold arcs; PE transpose; sel_T = relu(ha_T + gathered.T);
        label logits = sel_T.T @ W_lab + b_lab; logsumexp + gold as above.
      per-token ce = arc_ce + lab_ce -> [128, 2] output per core.
  - Host: mask by sentence length, global sum, /denom, *0.5.
"""

import sys
import numpy as np

for _p in ("/opt/trn_rl_repo", "/root/.axon_site/_ro/trn_rl_repo"):
    if _p not in sys.path:
        sys.path.append(_p)

import ml_dtypes

import concourse.bass as bass
from concourse import bacc
import concourse.mybir as mybir
import concourse.tile as tile
from concourse.bass import IndirectOffsetOnAxis
from concourse.bass_utils import run_bass_kernel_spmd
from concourse.masks import make_identity
from concourse.tile_rust import add_dep_helper

BF16 = mybir.dt.bfloat16
F32 = mybir.dt.float32
I32 = mybir.dt.int32
AF = mybir.ActivationFunctionType
ALU = mybir.AluOpType

B, L, D, H, TAGS = 16, 128, 512, 256, 45
NC_CORES = 8
NB = B // NC_CORES  # examples per core
J = L + 1  # head candidates (root + tokens)
HC = H // 128  # h chunks
DC = D // 128  # d chunks

_nb = ml_dtypes.bfloat16

_cached = {}

# j-loop relu engine split by (j*HC+hc) % 10: measured rates
# DVE ~163ns, GPSIMD ~?ns, ACT ~326ns per [128,128] tile
RELU_DVE = {0, 1, 2, 3, 4, 5, 6}
RELU_ACT = {7, 8, 9}
# GPSIMD shares SBUF ports with DVE - do NOT give it elementwise work


def _build_program():
    nc = bacc.Bacc("TRN2", target_bir_lowering=False, debug=False, num_devices=NC_CORES)

    # ---- I/O ----
    ctx_d = nc.dram_tensor("ctx_bf", [NB, 128, DC, 128], BF16, kind="ExternalInput")
    w1_d = nc.dram_tensor("w1_bf", [128, DC, H], BF16, kind="ExternalInput")
    wa_d = nc.dram_tensor("wa_bf", [128, HC, H], BF16, kind="ExternalInput")
    wb_d = nc.dram_tensor("wb_bf", [128, HC, H], BF16, kind="ExternalInput")
    warc_d = nc.dram_tensor("warc_bf", [128, HC, 1], BF16, kind="ExternalInput")
    wlab_d = nc.dram_tensor("wlab_bf", [128, HC, TAGS], BF16, kind="ExternalInput")
    b1_d = nc.dram_tensor("b1_f", [128, HC, 1], F32, kind="ExternalInput")
    bp_d = nc.dram_tensor("bp_f", [128, HC, 1], F32, kind="ExternalInput")
    blab_d = nc.dram_tensor("blab_f", [1, TAGS], F32, kind="ExternalInput")
    ones_d = nc.dram_tensor("ones_f", [1, 128], F32, kind="ExternalInput")
    root_d = nc.dram_tensor("root_bf", [128, HC, 1], BF16, kind="ExternalInput")
    iota_d = nc.dram_tensor("iota_f", [128, J], F32, kind="ExternalInput")
    arcs_d = nc.dram_tensor("arcs_f", [128, NB], F32, kind="ExternalInput")
    labs_d = nc.dram_tensor("labs_f", [128, NB], F32, kind="ExternalInput")
    gidx_d = nc.dram_tensor("gidx_i", [128, NB], I32, kind="ExternalInput")
    ce_d = nc.dram_tensor("ce_out", [128, NB], F32, kind="ExternalOutput")
    cbb_ds = [nc.dram_tensor(f"cbb_scratch{b}", [J, H], F32) for b in range(NB)]

    with tile.TileContext(nc) as tc:
        with (
            tc.tile_pool(name="consts", bufs=1) as consts,
            tc.tile_pool(name="bpool", bufs=2) as bpool,
            tc.tile_pool(name="small", bufs=4) as small,
            tc.tile_pool(name="pairs", bufs=16) as pairs_pool,
            tc.tile_pool(name="ps_big", bufs=2, space="PSUM") as ps_big,
            tc.tile_pool(name="ps_work", bufs=2, space="PSUM") as ps_work,
            tc.tile_pool(name="ps_lab", bufs=1, space="PSUM") as ps_lab,
        ):
            # ---- load constants ----
            w1_sb = consts.tile([128, DC, H], BF16)
            nc.sync.dma_start(out=w1_sb[:], in_=ctx_ap(w1_d))
            wa_sb = consts.tile([128, HC, H], BF16)
            nc.sync.dma_start(out=wa_sb[:], in_=ctx_ap(wa_d))
            wb_sb = consts.tile([128, HC, H], BF16)
            nc.sync.dma_start(out=wb_sb[:], in_=ctx_ap(wb_d))
            warc_sb = consts.tile([128, HC, 1], BF16)
            nc.sync.dma_start(out=warc_sb[:], in_=ctx_ap(warc_d))
            b1_sb = consts.tile([128, HC, 1], F32)
            nc.sync.dma_start(out=b1_sb[:], in_=ctx_ap(b1_d))
            bp_sb = consts.tile([128, HC, 1], F32)
            nc.sync.dma_start(out=bp_sb[:], in_=ctx_ap(bp_d))
            blab_sb = consts.tile([1, TAGS], F32)
            nc.sync.dma_start(out=blab_sb[:], in_=ctx_ap(blab_d))
            ones_sb = consts.tile([1, 128], F32)
            nc.sync.dma_start(out=ones_sb[:], in_=ctx_ap(ones_d))
            root_sb = consts.tile([128, HC, 1], BF16)
            nc.sync.dma_start(out=root_sb[:], in_=ctx_ap(root_d))
            iota_sb = consts.tile([128, J], F32)
            nc.sync.dma_start(out=iota_sb[:], in_=ctx_ap(iota_d))
            arcs_sb = consts.tile([128, NB], F32)
            nc.sync.dma_start(out=arcs_sb[:], in_=ctx_ap(arcs_d))
            labs_sb = consts.tile([128, NB], F32)
            nc.sync.dma_start(out=labs_sb[:], in_=ctx_ap(labs_d))
            gidx_sb = consts.tile([128, NB], I32)
            nc.sync.dma_start(out=gidx_sb[:], in_=ctx_ap(gidx_d))
            ident_sb = consts.tile([128, 128], F32)
            make_identity(nc, ident_sb[:])
            wlab_sb = consts.tile([128, HC, TAGS], BF16)
            nc.sync.dma_start(out=wlab_sb[:], in_=ctx_ap(wlab_d))
            ce_sb = consts.tile([128, NB], F32)

            for b in range(NB):
                # ---- ctx load (host pre-transposed): [128 d, DC, 128 i]
                ctxT = bpool.tile([128, DC, 128], BF16, tag="ctxT")
                nc.sync.dma_start(out=ctxT[:], in_=ctx_d.ap()[b])
                # ---- hidden (into cwr cols 1..128) ----
                cwrT = bpool.tile([128, HC, J], BF16, tag="cwrT")
                nc.vector.tensor_copy(cwrT[:, :, 0:1], root_sb[:])
                for hc in range(HC):
                    phw = ps_work.tile([128, H], F32, tag="work")
                    ph = phw[:, :128]
                    for dc in range(DC):
                        nc.tensor.matmul(
                            ph[:],
                            lhsT=w1_sb[:, dc, hc * 128 : (hc + 1) * 128],
                            rhs=ctxT[:, dc, :],
                            start=(dc == 0),
                            stop=(dc == DC - 1),
                        )
                    nc.scalar.activation(
                        cwrT[:, hc, 1:J], ph[:], AF.Relu, bias=b1_sb[:, hc, :]
                    )
                # ---- ha_T (+bp folded) ----
                haT = bpool.tile([128, HC, 128], BF16, tag="haT")
                for ac in range(HC):
                    paw = ps_work.tile([128, H], F32, tag="work")
                    pa = paw[:, :128]
                    for hc in range(HC):
                        nc.tensor.matmul(
                            pa[:],
                            lhsT=wa_sb[:, hc, ac * 128 : (ac + 1) * 128],
                            rhs=cwrT[:, hc, 1:J],
                            start=(hc == 0),
                            stop=(hc == HC - 1),
                        )
                    nc.vector.tensor_scalar(
                        out=haT[:, ac, :],
                        in0=pa[:],
                        scalar1=bp_sb[:, ac, :],
                        scalar2=None,
                        op0=ALU.add,
                    )
                # ---- cbb_T [128, 2, 129] f32 ----
                cbbT = bpool.tile([128, HC, J], F32, tag="cbbT")
                for bc in range(HC):
                    pc = ps_big.tile([128, J], F32, tag="pcb")
                    for hc in range(HC):
                        nc.tensor.matmul(
                            pc[:],
                            lhsT=wb_sb[:, hc, bc * 128 : (bc + 1) * 128],
                            rhs=cwrT[:, hc, :],
                            start=(hc == 0),
                            stop=(hc == HC - 1),
                        )
                    nc.scalar.copy(cbbT[:, bc, :], pc[:])
                # ---- cbb in [j, h] layout -> DRAM (for the gather) ----
                cj = bpool.tile([128, H], F32, tag="cj")
                pj = ps_work.tile([128, H], F32, tag="work")
                for hc in range(HC):
                    nc.tensor.matmul(
                        pj[:],
                        lhsT=cwrT[:, hc, 0:128],
                        rhs=wb_sb[:, hc, :],
                        start=(hc == 0),
                        stop=(hc == HC - 1),
                    )
                nc.vector.tensor_copy(cj[:], pj[:])
                st1 = nc.sync.dma_start(
                    out=cbb_ds[b].ap()[0:128, :], in_=cj[:]
                )
                cjl = bpool.tile([1, H], F32, tag="cjl")
                pjl = ps_lab.tile([1, H], F32, tag="pjl")
                for hc in range(HC):
                    nc.tensor.matmul(
                        pjl[:],
                        lhsT=cwrT[:, hc, 128:129],
                        rhs=wb_sb[:, hc, :],
                        start=(hc == 0),
                        stop=(hc == HC - 1),
                    )
                nc.vector.tensor_copy(cjl[:], pjl[:])
                st2 = nc.sync.dma_start(
                    out=cbb_ds[b].ap()[128:J, :], in_=cjl[:]
                )
                # ---- gather cbb rows at gold arcs ----
                csel = bpool.tile([128, H], F32, tag="csel")
                g = nc.gpsimd.indirect_dma_start(
                    out=csel[:],
                    out_offset=None,
                    in_=cbb_ds[b].ap(),
                    in_offset=IndirectOffsetOnAxis(ap=gidx_sb[:, b : b + 1], axis=0),
                )
                add_dep_helper(g.ins, st1.ins, sync=True, reason="cbb store->gather")
                add_dep_helper(g.ins, st2.ins, sync=True, reason="cbb store->gather")

                # ---- the quadratic j-loop ----
                arc_ps = ps_big.tile([128, J], F32, tag="arc")
                for j in range(J):
                    for hc in range(HC):
                        pt = pairs_pool.tile([128, 128], BF16, tag="pairs")
                        k = (j * HC + hc) % 10
                        if k in RELU_ACT:
                            nc.scalar.activation(
                                pt[:],
                                haT[:, hc, :],
                                AF.Relu,
                                bias=cbbT[:, hc, j : j + 1],
                            )
                        else:
                            nc.vector.tensor_scalar(
                                out=pt[:],
                                in0=haT[:, hc, :],
                                scalar1=cbbT[:, hc, j : j + 1],
                                scalar2=0.0,
                                op0=ALU.add,
                                op1=ALU.max,
                            )
                        nc.tensor.matmul(
                            arc_ps[:, j : j + 1],
                            lhsT=pt[:],
                            rhs=warc_sb[:, hc, :],
                            start=(hc == 0),
                            stop=(hc == HC - 1),
                        )

                # ---- arc logsumexp + gold ----
                negm = small.tile([128, 1], F32, tag="negm")
                nc.vector.tensor_reduce(
                    negm[:], arc_ps[:], axis=mybir.AxisListType.X, op=ALU.max,
                    negate=True,
                )
                et = bpool.tile([128, J], F32, tag="et")
                es = small.tile([128, 1], F32, tag="es")
                nc.scalar.activation(
                    et[:], arc_ps[:], AF.Exp, bias=negm[:], accum_out=es[:]
                )
                lns = small.tile([128, 1], F32, tag="lns")
                nc.scalar.activation(lns[:], es[:], AF.Ln)
                golda = small.tile([128, 1], F32, tag="golda")
                sc2 = bpool.tile([128, J], F32, tag="sc2")
                nc.vector.scalar_tensor_tensor(
                    out=sc2[:],
                    in0=iota_sb[:],
                    scalar=arcs_sb[:, b : b + 1],
                    op0=ALU.is_equal,
                    in1=arc_ps[:],
                    op1=ALU.mult,
                    accum_out=golda[:],
                )
                cea = small.tile([128, 1], F32, tag="cea")
                nc.vector.tensor_sub(cea[:], lns[:], negm[:])
                nc.vector.tensor_sub(cea[:], cea[:], golda[:])

                # ---- label path ----
                selT = bpool.tile([128, HC, 128], BF16, tag="selT")
                for hc in range(HC):
                    ptrw = ps_work.tile([128, H], F32, tag="work")
                    ptr = ptrw[:, :128]
                    nc.tensor.transpose(
                        ptr[:], csel[:, hc * 128 : (hc + 1) * 128], ident_sb[:]
                    )
                    tmp = bpool.tile([128, 128], F32, tag="seltmp")
                    nc.vector.tensor_add(tmp[:], ptr[:], haT[:, hc, :])
                    nc.vector.tensor_scalar(
                        out=selT[:, hc, :], in0=tmp[:], scalar1=0.0, op0=ALU.max,
                        scalar2=None,
                    )
                lab_ps = ps_lab.tile([128, TAGS], F32, tag="lab")
                for hc in range(HC):
                    nc.tensor.matmul(
                        lab_ps[:],
                        lhsT=selT[:, hc, :],
                        rhs=wlab_sb[:, hc, :],
                        start=(hc == 0),
                        stop=False,
                    )
                nc.tensor.matmul(
                    lab_ps[:], lhsT=ones_sb[:], rhs=blab_sb[:], start=False, stop=True
                )
                negml = small.tile([128, 1], F32, tag="negml")
                nc.vector.tensor_reduce(
                    negml[:], lab_ps[:], axis=mybir.AxisListType.X, op=ALU.max,
                    negate=True,
                )
                etl = bpool.tile([128, TAGS], F32, tag="etl")
                esl = small.tile([128, 1], F32, tag="esl")
                nc.scalar.activation(
                    etl[:], lab_ps[:], AF.Exp, bias=negml[:], accum_out=esl[:]
                )
                lnsl = small.tile([128, 1], F32, tag="lnsl")
                nc.scalar.activation(lnsl[:], esl[:], AF.Ln)
                goldl = small.tile([128, 1], F32, tag="goldl")
                sc2l = bpool.tile([128, TAGS], F32, tag="sc2l")
                nc.vector.scalar_tensor_tensor(
                    out=sc2l[:],
                    in0=iota_sb[:, :TAGS],
                    scalar=labs_sb[:, b : b + 1],
                    op0=ALU.is_equal,
                    in1=lab_ps[:],
                    op1=ALU.mult,
                    accum_out=goldl[:],
                )
                cel = small.tile([128, 1], F32, tag="cel")
                nc.vector.tensor_sub(cel[:], lnsl[:], negml[:])
                nc.vector.tensor_sub(cel[:], cel[:], goldl[:])

                nc.vector.tensor_add(ce_sb[:, b : b + 1], cea[:], cel[:])

            nc.sync.dma_start(out=ce_d.ap(), in_=ce_sb[:])

    nc.compile()
    return nc


def ctx_ap(d):
    return d.ap()


def _prep_in_maps(inputs):
    ctx = np.asarray(inputs["contextualized"], np.float32)
    arcs = np.asarray(inputs["desired_arcs"], np.int32)
    labs = np.asarray(inputs["desired_labels"], np.int32)
    W1 = np.asarray(inputs["W1"], np.float32)
    b1 = np.asarray(inputs["b1"], np.float32)
    root = np.asarray(inputs["root"], np.float32)
    Wp = np.asarray(inputs["Wp"], np.float32)
    bp = np.asarray(inputs["bp"], np.float32)
    W_arc = np.asarray(inputs["W_arc"], np.float32)
    W_lab = np.asarray(inputs["W_lab"], np.float32)
    b_lab = np.asarray(inputs["b_lab"], np.float32)

    def chunked(w, nch):  # [nch*128, X] -> [128, nch, X]
        return np.ascontiguousarray(
            w.reshape(nch, 128, -1).transpose(1, 0, 2)
        )

    w1_bf = chunked(W1, DC).astype(_nb)
    wa_bf = chunked(Wp[:H], HC).astype(_nb)
    wb_bf = chunked(Wp[H:], HC).astype(_nb)
    warc_bf = chunked(W_arc, HC).astype(_nb)
    wlab_bf = chunked(W_lab, HC).astype(_nb)
    b1_f = chunked(b1[:, None], HC)
    bp_f = chunked(bp[:, None], HC)
    root_bf = chunked(root[:, None], HC).astype(_nb)
    blab_f = np.ascontiguousarray(b_lab[None, :], dtype=np.float32)
    ones_f = np.ones((1, 128), np.float32)
    iota_f = np.broadcast_to(
        np.arange(J, dtype=np.float32)[None, :], (128, J)
    ).copy()

    in_maps = []
    for c in range(NC_CORES):
        bs = slice(c * NB, (c + 1) * NB)
        arcs_c = arcs[bs]  # [NB, 128]
        in_maps.append(
            {
                "ctx_bf": np.ascontiguousarray(
                    ctx[bs].reshape(NB, L, DC, 128).transpose(0, 3, 2, 1)
                ).astype(_nb),
                "w1_bf": w1_bf,
                "wa_bf": wa_bf,
                "wb_bf": wb_bf,
                "warc_bf": warc_bf,
                "wlab_bf": wlab_bf,
                "b1_f": b1_f,
                "bp_f": bp_f,
                "blab_f": blab_f,
                "ones_f": ones_f,
                "root_bf": root_bf,
                "iota_f": iota_f,
                "arcs_f": np.ascontiguousarray(arcs_c.T, np.float32),
                "labs_f": np.ascontiguousarray(labs[bs].T, np.float32),
                "gidx_i": np.ascontiguousarray(arcs_c.T).astype(np.int32),
            }
        )
    return in_maps


def kernel(**inputs) -> np.ndarray:
    if "nc" not in _cached:
        _cached["nc"] = _build_program()
    nc = _cached["nc"]
    in_maps = _prep_in_maps(inputs)
    res = run_bass_kernel_spmd(nc, in_maps, list(range(NC_CORES)))
    ce = np.concatenate([r["ce_out"] for r in res.results], axis=1)  # [128, B]
    lens = np.asarray(inputs["sentence_lengths"], np.int32)  # [B]
    mask = (np.arange(L)[None, :] < lens[:, None]).astype(np.float32)  # [B, L]
    total = float(np.sum(ce.T.astype(np.float64) * mask))
    denom = max(float(mask.sum()), 1.0)
    return np.array(0.5 * total / denom, dtype=np.float32)
